# revision 16
# baseline (speedup 1.0000x reference)
"""Multi-head self-attention (B=4, T=2048, C=768, H=12) on 8 trn2 NeuronCores.

Sharding: core c -> batch b=c//2, head-group g=c%2 (6 heads each).
Each core computes its 6 heads' attention and a partial output projection
(contraction over its 384 ctx dims). Host sums the 2 partials per batch
and adds the bias.

Per-core kernel (all matmuls in float32r, 1 cycle/row on the PE):
  Xb[2048,768] -> X^T (PE transpose) -> qT,kT,vT[384,2048] projections
  v_aug[t][128, 2x65]: v rows with a ones column (softmax denominators
  come out of the ctx matmul for free).
  scores^T chunk = kT_chunk.T @ qT  -> exp on ACT (scale folded in)
  ctx^T[65,Tq]  += v_aug.T @ P^T    (row 64 = sum of exp)
  normalize: R = ones x recip(sums) (PE outer product), ctxT = ctx_u * R
  out[t] = sum_m ctxT[m].T @ Wo[m]  -> DMA out (partial, pre-bias)

PSUM budget (8 banks): tp1 2 (X^T, early) / psproj 1 + tp2 1 (proj)
/ sps 4 + cps 2 (attention; R borrows an sps slot) / pso 2 (outproj).

KERNEL_REPEAT=N builds the body N times (for overhead-cancelling timing).
"""
import sys
import os

sys.path.insert(0, "/opt/trn_rl_repo")

import numpy as np

P = 128
T = 2048
C = 768
HD = 384          # per-core head columns (6 heads x 64)
D = 64
NT = T // P       # 16 T chunks of 128
KC = C // P       # 6 contraction chunks for C
MC = HD // P      # 3 chunks of head dims
NH = 6            # heads per core
HALF = 1024       # T_q blocking for the attention inner loop
VW = 2 * D + 2    # 130: v_aug column block per T chunk (2 heads x 65)

_cache = {}


def _build(repeat=1):
    import concourse.bacc as bacc
    import concourse.mybir as mybir
    import concourse.tile as tile
    from concourse.masks import make_identity
    from contextlib import ExitStack

    F32 = mybir.dt.float32
    F32R = mybir.dt.float32r
    AF = mybir.ActivationFunctionType
    ALU = mybir.AluOpType

    nc = bacc.Bacc("TRN2", target_bir_lowering=False, debug=False)
    x = nc.dram_tensor("x", [T, C], F32, kind="ExternalInput").ap()
    wq = nc.dram_tensor("wq", [C, HD], F32, kind="ExternalInput").ap()
    wk = nc.dram_tensor("wk", [C, HD], F32, kind="ExternalInput").ap()
    wv = nc.dram_tensor("wv", [C, HD], F32, kind="ExternalInput").ap()
    wo = nc.dram_tensor("wo", [HD, C], F32, kind="ExternalInput").ap()
    out = nc.dram_tensor("out", [T, C], F32, kind="ExternalOutput").ap()

    def emit(pfx, tc, pools):
        (ident, ones_r), big, wrp, vap, work, outp, norm = pools

        # ---- X^T via PE transpose: xt[kc] = X[:, 128kc:+128].T  (f32r)
        # (emitted first so the X DMAs lead the queue and attention's
        # upstream starts immediately)
        xt = [big.tile([P, T], F32R, name=f"{pfx}xt{kc}", tag="big2048") for kc in range(KC)]
        with tc.tile_pool(name=pfx + "xrp", bufs=5) as xrp, \
             tc.tile_pool(name=pfx + "tp1", bufs=2, space="PSUM") as tp1:
            for tq in range(NT // 4):      # groups of 4 T chunks
                xrs = []
                for i in range(4):
                    t_i = 4 * tq + i
                    xr = xrp.tile([P, C], F32, name=f"{pfx}xr{t_i}", tag="xr")
                    nc.sync.dma_start(xr[:], x[P * t_i:P * (t_i + 1), :])
                    xrs.append(xr)
                for kc in range(KC):
                    tp = tp1.tile([P, 512], F32, name=f"{pfx}tp_{tq}_{kc}", tag="tp")
                    for i in range(4):
                        nc.tensor.transpose(tp[:, P * i:P * (i + 1)], xrs[i][:, P * kc:P * (kc + 1)], ident[:])
                    nc.vector.tensor_copy(xt[kc][:, 512 * tq:512 * (tq + 1)], tp[:])

        # ---- load weights, cast to f32r (rounding producer = DVE copy)
        w_r = {}
        wo_r = []
        with tc.tile_pool(name=pfx + "wstage", bufs=2) as wstage:
            for nm, src in (("q", wq), ("k", wk), ("v", wv)):
                for kc in range(KC):
                    st = wstage.tile([P, HD], F32, name=f"{pfx}wst_{nm}{kc}", tag="wst")
                    nc.sync.dma_start(st[:], src[P * kc:P * (kc + 1), :])
                    t_r = wrp.tile([P, HD], F32R, name=f"{pfx}w_{nm}{kc}", tag=f"w_{nm}{kc}")
                    nc.vector.tensor_copy(t_r[:], st[:])
                    w_r[nm, kc] = t_r
            for m in range(MC):
                st = wstage.tile([P, C], F32, name=f"{pfx}wst_o{m}", tag="wsto")
                nc.sync.dma_start(st[:], wo[P * m:P * (m + 1), :])
                t_r = wrp.tile([P, C], F32R, name=f"{pfx}wo_{m}", tag=f"wo_{m}")
                nc.vector.tensor_copy(t_r[:], st[:])
                wo_r.append(t_r)

        # ---- projections, per head-dim chunk m: q, k, v(+v_aug)
        qT = [big.tile([P, T], F32R, name=f"{pfx}qT{m}", tag="big2048") for m in range(MC)]
        kT = [big.tile([P, T], F32R, name=f"{pfx}kT{m}", tag="big2048") for m in range(MC)]
        va = [vap.tile([P, NT * VW], F32R, name=f"{pfx}va{m}", tag=f"va{m}") for m in range(MC)]

        with tc.tile_pool(name=pfx + "psproj", bufs=1, space="PSUM") as psproj, \
             tc.tile_pool(name=pfx + "tp2", bufs=1, space="PSUM") as tp2, \
             tc.tile_pool(name=pfx + "vtp", bufs=1) as vtp:
            for m in range(MC):
                for nm in ("q", "k", "v"):
                    dest = {"q": qT, "k": kT, "v": None}[nm]
                    if dest is None:
                        vT = vtp.tile([P, T], F32, name=f"{pfx}vT{m}", tag="vT")
                    for n in range(4):
                        ps = psproj.tile([P, 512], F32, name=f"{pfx}ps_{nm}{m}{n}", tag="ps")
                        for kc in range(KC):
                            nc.tensor.matmul(
                                ps[:],
                                w_r[nm, kc][:, P * m:P * (m + 1)],
                                xt[kc][:, 512 * n:512 * (n + 1)],
                                start=(kc == 0), stop=(kc == KC - 1),
                            )
                        if dest is not None:
                            nc.vector.tensor_copy(dest[m][:, 512 * n:512 * (n + 1)], ps[:])
                        else:
                            nc.vector.tensor_copy(vT[:, 512 * n:512 * (n + 1)], ps[:])
                # v -> v_aug (transpose + ones column)
                vav = va[m][:].rearrange("p (t g c) -> p t g c", t=NT, g=2)
                for t_i in range(NT):
                    tp = tp2.tile([P, P], F32, name=f"{pfx}vtp{m}_{t_i}", tag="vtp")
                    nc.tensor.transpose(tp[:], vT[:, P * t_i:P * (t_i + 1)], ident[:])
                    nc.vector.tensor_copy(
                        vav[:, t_i, :, 0:D],
                        tp[:].rearrange("p (g c) -> p g c", g=2),
                    )
                nc.vector.tensor_scalar(
                    vav[:, :, :, D:D + 1],
                    ident[:, 0:2 * NT].rearrange("p (t g c) -> p t g c", t=NT, g=2, c=1),
                    0.0, 1.0, ALU.mult, ALU.add,
                )

        # ---- attention (half outer so out-proj can chase) + out-proj
        ctxT = [big.tile([P, T], F32R, name=f"{pfx}ctxT{m}", tag="big2048") for m in range(MC)]
        with tc.tile_pool(name=pfx + "sps", bufs=2, space="PSUM") as spsp, \
             tc.tile_pool(name=pfx + "cps", bufs=1, space="PSUM") as cpsp, \
             tc.tile_pool(name=pfx + "pso", bufs=2, space="PSUM") as psop:
            for half in range(T // HALF):
                q0 = HALF * half
                for h in range(NH):
                    m, par = divmod(h, 2)
                    qh = qT[m][D * par:D * (par + 1), :]
                    kh = kT[m][D * par:D * (par + 1), :]
                    vav = va[m][:].rearrange("p (t g c) -> p t g c", t=NT, g=2)
                    cps = cpsp.tile([D + 1, HALF], F32, name=f"{pfx}cps{h}_{half}", tag="cps")
                    for j in range(NT):
                        sps = spsp.tile([P, HALF], F32, name=f"{pfx}sps{h}{half}{j}", tag="sps")
                        for u in range(HALF // 512):
                            nc.tensor.matmul(
                                sps[:, 512 * u:512 * (u + 1)],
                                kh[:, P * j:P * (j + 1)],
                                qh[:, q0 + 512 * u:q0 + 512 * (u + 1)],
                                start=True, stop=True,
                            )
                        pt = work.tile([P, HALF], F32R, name=f"{pfx}pt{h}{half}{j}", tag="pt")
                        nc.scalar.activation(pt[:], sps[:], AF.Exp, scale=float(D) ** -0.5)
                        for u in range(HALF // 512):
                            nc.tensor.matmul(
                                cps[:, 512 * u:512 * (u + 1)],
                                vav[:, j, par, :],
                                pt[:, 512 * u:512 * (u + 1)],
                                start=(j == 0), stop=(j == NT - 1),
                            )
                    # normalize this [64, HALF] ctx^T block.  NB: the sums row
                    # must be staged to a partition-0 tile -- custom DVE ops
                    # (reciprocal_approx_fast) misread partition-offset inputs.
                    cu = norm.tile([D, HALF], F32, name=f"{pfx}cu{h}{half}", tag="cu")
                    nc.vector.tensor_copy(cu[:], cps[0:D, :])
                    s_sb = norm.tile([1, HALF], F32, name=f"{pfx}ssb{h}{half}", tag="ssb")
                    nc.vector.tensor_copy(s_sb[:], cps[D:D + 1, :])
                    rr = norm.tile([1, HALF], F32, name=f"{pfx}rr{h}{half}", tag="rr")
                    nc.vector.reciprocal_approx_fast(rr[:], s_sb[:])
                    rr_r = norm.tile([1, HALF], F32R, name=f"{pfx}rrr{h}{half}", tag="rrr")
                    nc.vector.tensor_copy(rr_r[:], rr[:])
                    for u in range(HALF // 512):
                        rps = spsp.tile([D, 512], F32, name=f"{pfx}rps{h}{half}{u}", tag="sps")
                        nc.tensor.matmul(rps[:], ones_r[:], rr_r[:, 512 * u:512 * (u + 1)],
                                         start=True, stop=True)
                        nc.vector.tensor_mul(
                            ctxT[m][D * par:D * (par + 1), q0 + 512 * u:q0 + 512 * (u + 1)],
                            cu[:, 512 * u:512 * (u + 1)],
                            rps[:],
                        )
                # out-proj for the T rows of this half
                for t_i in range(NT * half // (T // HALF), NT * (half + 1) // (T // HALF)):
                    psA = psop.tile([P, 512], F32, name=f"{pfx}psA{t_i}", tag="pso")
                    psB = psop.tile([P, C - 512], F32, name=f"{pfx}psB{t_i}", tag="pso")
                    for m in range(MC):
                        nc.tensor.matmul(psA[:], ctxT[m][:, P * t_i:P * (t_i + 1)],
                                         wo_r[m][:, 0:512], start=(m == 0), stop=(m == MC - 1))
                        nc.tensor.matmul(psB[:], ctxT[m][:, P * t_i:P * (t_i + 1)],
                                         wo_r[m][:, 512:C], start=(m == 0), stop=(m == MC - 1))
                    ob = outp.tile([P, C], F32, name=f"{pfx}ob{t_i}", tag="ob")
                    nc.vector.tensor_copy(ob[:, 0:512], psA[:])
                    nc.vector.tensor_copy(ob[:, 512:C], psB[:])
                    nc.sync.dma_start(out[P * t_i:P * (t_i + 1), :], ob[:])

    with tile.TileContext(nc) as tc, ExitStack() as ctx:
        consts = ctx.enter_context(tc.tile_pool(name="consts", bufs=1))
        ident = consts.tile([P, P], F32)
        make_identity(nc, ident)
        ones_r = consts.tile([1, D], F32R)
        nc.vector.tensor_scalar(ones_r[:], ident[0:1, 0:D], 0.0, 1.0, ALU.mult, ALU.add)

        big = ctx.enter_context(tc.tile_pool(name="big", bufs=12))
        wrp = ctx.enter_context(tc.tile_pool(name="wrp", bufs=1))
        vap = ctx.enter_context(tc.tile_pool(name="vap", bufs=1))
        work = ctx.enter_context(tc.tile_pool(name="work", bufs=2))
        outp = ctx.enter_context(tc.tile_pool(name="outp", bufs=2))
        norm = ctx.enter_context(tc.tile_pool(name="norm", bufs=1))
        pools = ((ident, ones_r), big, wrp, vap, work, outp, norm)
        for rep in range(repeat):
            emit(f"r{rep}_", tc, pools)

    nc.compile()
    return nc


def kernel(X, Wq, Wk, Wv, Wo, bo):
    from concourse import bass_utils

    if "nc" not in _cache:
        _cache["nc"] = _build(int(os.environ.get("KERNEL_REPEAT", "1")))
    nc = _cache["nc"]

    X = np.asarray(X, dtype=np.float32)
    in_maps = []
    for c in range(8):
        b, g = divmod(c, 2)
        sl = slice(HD * g, HD * (g + 1))
        in_maps.append({
            "x": np.ascontiguousarray(X[b]),
            "wq": np.ascontiguousarray(np.asarray(Wq, np.float32)[:, sl]),
            "wk": np.ascontiguousarray(np.asarray(Wk, np.float32)[:, sl]),
            "wv": np.ascontiguousarray(np.asarray(Wv, np.float32)[:, sl]),
            "wo": np.ascontiguousarray(np.asarray(Wo, np.float32)[sl, :]),
        })
    res = bass_utils.run_bass_kernel_spmd(nc, in_maps, core_ids=list(range(8)))
    outf = np.empty((4, T, C), np.float32)
    bo = np.asarray(bo, np.float32)
    for b in range(4):
        outf[b] = res.results[2 * b]["out"] + res.results[2 * b + 1]["out"] + bo
    return outf


# revision 17
# speedup vs baseline: 1.5075x; 1.5075x over previous
"""Multi-head self-attention (B=4, T=2048, C=768, H=12) on 8 trn2 NeuronCores.

Sharding: core c -> batch b=c//2, head-group g=c%2 (6 heads each).
Each core computes its 6 heads' attention and a partial output projection
(contraction over its 384 ctx dims). Host sums the 2 partials per batch
and adds the bias.

Per-core kernel (all matmuls in float32r, 1 cycle/row on the PE):
  Xb[2048,768] -> X^T (PE transpose) -> qT,kT,vT[384,2048] projections
  v_aug[t][128, 2x65]: v rows with a ones column (softmax denominators
  come out of the ctx matmul for free).
  scores^T chunk = kT_chunk.T @ qT  -> exp on ACT (scale folded in)
  ctx^T[65,Tq]  += v_aug.T @ P^T    (row 64 = sum of exp)
  normalize: R = ones x recip(sums) (PE outer product), ctxT = ctx_u * R
  out[t] = sum_m ctxT[m].T @ Wo[m]  -> DMA out (partial, pre-bias)

PSUM budget (8 banks): tp1 2 (X^T, early) / psproj 1 + tp2 1 (proj)
/ sps 4 + cps 2 (attention; R borrows an sps slot) / pso 2 (outproj).

KERNEL_REPEAT=N builds the body N times (for overhead-cancelling timing).
"""
import sys
import os

sys.path.insert(0, "/opt/trn_rl_repo")

import numpy as np

P = 128
T = 2048
C = 768
HD = 384          # per-core head columns (6 heads x 64)
D = 64
NT = T // P       # 16 T chunks of 128
KC = C // P       # 6 contraction chunks for C
MC = HD // P      # 3 chunks of head dims
NH = 6            # heads per core
HALF = 1024       # T_q blocking for the attention inner loop
VW = 2 * D + 2    # 130: v_aug column block per T chunk (2 heads x 65)

_cache = {}


def _build(repeat=1):
    import concourse.bacc as bacc
    import concourse.mybir as mybir
    import concourse.tile as tile
    from concourse.masks import make_identity
    from contextlib import ExitStack

    F32 = mybir.dt.float32
    F32R = mybir.dt.float32r
    AF = mybir.ActivationFunctionType
    ALU = mybir.AluOpType

    nc = bacc.Bacc("TRN2", target_bir_lowering=False, debug=False)
    x = nc.dram_tensor("x", [T, C], F32, kind="ExternalInput").ap()
    wq = nc.dram_tensor("wq", [C, HD], F32, kind="ExternalInput").ap()
    wk = nc.dram_tensor("wk", [C, HD], F32, kind="ExternalInput").ap()
    wv = nc.dram_tensor("wv", [C, HD], F32, kind="ExternalInput").ap()
    wo = nc.dram_tensor("wo", [HD, C], F32, kind="ExternalInput").ap()
    out = nc.dram_tensor("out", [T, C], F32, kind="ExternalOutput").ap()

    def emit(pfx, tc, pools):
        (ident, ones_r), big, wrp, vap, work, outp, norm = pools

        # ---- X^T via PE transpose: xt[kc] = X[:, 128kc:+128].T  (f32r)
        # (emitted first so the X DMAs lead the queue and attention's
        # upstream starts immediately)
        xt = [big.tile([P, T], F32R, name=f"{pfx}xt{kc}", tag="big2048") for kc in range(KC)]
        with tc.tile_pool(name=pfx + "xrp", bufs=5) as xrp, \
             tc.tile_pool(name=pfx + "tp1", bufs=2, space="PSUM") as tp1:
            for tq in range(NT // 4):      # groups of 4 T chunks
                xrs = []
                for i in range(4):
                    t_i = 4 * tq + i
                    xr = xrp.tile([P, C], F32, name=f"{pfx}xr{t_i}", tag="xr")
                    nc.sync.dma_start(xr[:], x[P * t_i:P * (t_i + 1), :])
                    xrs.append(xr)
                for kc in range(KC):
                    tp = tp1.tile([P, 512], F32, name=f"{pfx}tp_{tq}_{kc}", tag="tp")
                    for i in range(4):
                        nc.tensor.transpose(tp[:, P * i:P * (i + 1)], xrs[i][:, P * kc:P * (kc + 1)], ident[:])
                    nc.vector.tensor_copy(xt[kc][:, 512 * tq:512 * (tq + 1)], tp[:])

        # ---- load weights, cast to f32r (rounding producer = DVE copy)
        w_r = {}
        wo_r = []
        with tc.tile_pool(name=pfx + "wstage", bufs=2) as wstage:
            for nm, src in (("q", wq), ("k", wk), ("v", wv)):
                for kc in range(KC):
                    st = wstage.tile([P, HD], F32, name=f"{pfx}wst_{nm}{kc}", tag="wst")
                    nc.sync.dma_start(st[:], src[P * kc:P * (kc + 1), :])
                    t_r = wrp.tile([P, HD], F32R, name=f"{pfx}w_{nm}{kc}", tag=f"w_{nm}{kc}")
                    nc.vector.tensor_copy(t_r[:], st[:])
                    w_r[nm, kc] = t_r
            for m in range(MC):
                st = wstage.tile([P, C], F32, name=f"{pfx}wst_o{m}", tag="wsto")
                nc.sync.dma_start(st[:], wo[P * m:P * (m + 1), :])
                t_r = wrp.tile([P, C], F32R, name=f"{pfx}wo_{m}", tag=f"wo_{m}")
                nc.vector.tensor_copy(t_r[:], st[:])
                wo_r.append(t_r)

        # ---- projections, per head-dim chunk m: q, k, v(+v_aug)
        qT = [big.tile([P, T], F32R, name=f"{pfx}qT{m}", tag="big2048") for m in range(MC)]
        kT = [big.tile([P, T], F32R, name=f"{pfx}kT{m}", tag="big2048") for m in range(MC)]
        va = [vap.tile([P, NT * VW], F32R, name=f"{pfx}va{m}", tag=f"va{m}") for m in range(MC)]

        with tc.tile_pool(name=pfx + "psproj", bufs=4, space="PSUM") as psproj, \
             tc.tile_pool(name=pfx + "tp2", bufs=2, space="PSUM") as tp2, \
             tc.tile_pool(name=pfx + "vtp", bufs=1) as vtp:
            for m in range(MC):
                for nm in ("q", "k", "v"):
                    dest = {"q": qT, "k": kT, "v": None}[nm]
                    if dest is None:
                        vT = vtp.tile([P, T], F32, name=f"{pfx}vT{m}", tag="vT")
                    for n in range(4):
                        ps = psproj.tile([P, 512], F32, name=f"{pfx}ps_{nm}{m}{n}", tag="ps")
                        for kc in range(KC):
                            nc.tensor.matmul(
                                ps[:],
                                w_r[nm, kc][:, P * m:P * (m + 1)],
                                xt[kc][:, 512 * n:512 * (n + 1)],
                                start=(kc == 0), stop=(kc == KC - 1),
                            )
                        if dest is not None:
                            nc.vector.tensor_copy(dest[m][:, 512 * n:512 * (n + 1)], ps[:])
                        else:
                            nc.vector.tensor_copy(vT[:, 512 * n:512 * (n + 1)], ps[:])
                # v -> v_aug (transpose + ones column)
                vav = va[m][:].rearrange("p (t g c) -> p t g c", t=NT, g=2)
                for t_i in range(NT):
                    tp = tp2.tile([P, P], F32, name=f"{pfx}vtp{m}_{t_i}", tag="vtp")
                    nc.tensor.transpose(tp[:], vT[:, P * t_i:P * (t_i + 1)], ident[:])
                    nc.vector.tensor_copy(
                        vav[:, t_i, :, 0:D],
                        tp[:].rearrange("p (g c) -> p g c", g=2),
                    )
                nc.vector.tensor_scalar(
                    vav[:, :, :, D:D + 1],
                    ident[:, 0:2 * NT].rearrange("p (t g c) -> p t g c", t=NT, g=2, c=1),
                    0.0, 1.0, ALU.mult, ALU.add,
                )

        # ---- attention (half outer so out-proj can chase) + out-proj
        ctxT = [big.tile([P, T], F32R, name=f"{pfx}ctxT{m}", tag="big2048") for m in range(MC)]
        with tc.tile_pool(name=pfx + "sps", bufs=2, space="PSUM") as spsp, \
             tc.tile_pool(name=pfx + "cps", bufs=1, space="PSUM") as cpsp, \
             tc.tile_pool(name=pfx + "rps", bufs=1, space="PSUM") as rpsp, \
             tc.tile_pool(name=pfx + "pso", bufs=1, space="PSUM") as psop:
            for half in range(T // HALF):
                q0 = HALF * half
                for h in range(NH):
                    m, par = divmod(h, 2)
                    qh = qT[m][D * par:D * (par + 1), :]
                    kh = kT[m][D * par:D * (par + 1), :]
                    vav = va[m][:].rearrange("p (t g c) -> p t g c", t=NT, g=2)
                    cps = cpsp.tile([D + 1, HALF], F32, name=f"{pfx}cps{h}_{half}", tag="cps")
                    for j in range(NT):
                        sps = spsp.tile([P, HALF], F32, name=f"{pfx}sps{h}{half}{j}", tag="sps")
                        for u in range(HALF // 512):
                            nc.tensor.matmul(
                                sps[:, 512 * u:512 * (u + 1)],
                                kh[:, P * j:P * (j + 1)],
                                qh[:, q0 + 512 * u:q0 + 512 * (u + 1)],
                                start=True, stop=True,
                            )
                        pt = work.tile([P, HALF], F32R, name=f"{pfx}pt{h}{half}{j}", tag="pt")
                        nc.scalar.activation(pt[:], sps[:], AF.Exp, scale=float(D) ** -0.5)
                        for u in range(HALF // 512):
                            nc.tensor.matmul(
                                cps[:, 512 * u:512 * (u + 1)],
                                vav[:, j, par, :],
                                pt[:, 512 * u:512 * (u + 1)],
                                start=(j == 0), stop=(j == NT - 1),
                            )
                    # normalize this [64, HALF] ctx^T block.  NB: the sums row
                    # must be staged to a partition-0 tile -- custom DVE ops
                    # (reciprocal_approx_fast) misread partition-offset inputs.
                    cu = norm.tile([D, HALF], F32, name=f"{pfx}cu{h}{half}", tag="cu")
                    nc.vector.tensor_copy(cu[:], cps[0:D, :])
                    s_sb = norm.tile([1, HALF], F32, name=f"{pfx}ssb{h}{half}", tag="ssb")
                    nc.vector.tensor_copy(s_sb[:], cps[D:D + 1, :])
                    rr = norm.tile([1, HALF], F32, name=f"{pfx}rr{h}{half}", tag="rr")
                    nc.vector.reciprocal_approx_fast(rr[:], s_sb[:])
                    rr_r = norm.tile([1, HALF], F32R, name=f"{pfx}rrr{h}{half}", tag="rrr")
                    nc.vector.tensor_copy(rr_r[:], rr[:])
                    for u in range(HALF // 512):
                        rps = rpsp.tile([D, 512], F32, name=f"{pfx}rps{h}{half}{u}", tag="rps")
                        nc.tensor.matmul(rps[:], ones_r[:], rr_r[:, 512 * u:512 * (u + 1)],
                                         start=True, stop=True)
                        nc.vector.tensor_mul(
                            ctxT[m][D * par:D * (par + 1), q0 + 512 * u:q0 + 512 * (u + 1)],
                            cu[:, 512 * u:512 * (u + 1)],
                            rps[:],
                        )
                # out-proj for the T rows of this half
                for t_i in range(NT * half // (T // HALF), NT * (half + 1) // (T // HALF)):
                    psA = psop.tile([P, 512], F32, name=f"{pfx}psA{t_i}", tag="pso")
                    psB = psop.tile([P, C - 512], F32, name=f"{pfx}psB{t_i}", tag="pso")
                    for m in range(MC):
                        nc.tensor.matmul(psA[:], ctxT[m][:, P * t_i:P * (t_i + 1)],
                                         wo_r[m][:, 0:512], start=(m == 0), stop=(m == MC - 1))
                        nc.tensor.matmul(psB[:], ctxT[m][:, P * t_i:P * (t_i + 1)],
                                         wo_r[m][:, 512:C], start=(m == 0), stop=(m == MC - 1))
                    ob = outp.tile([P, C], F32, name=f"{pfx}ob{t_i}", tag="ob")
                    nc.vector.tensor_copy(ob[:, 0:512], psA[:])
                    nc.vector.tensor_copy(ob[:, 512:C], psB[:])
                    nc.sync.dma_start(out[P * t_i:P * (t_i + 1), :], ob[:])

    with tile.TileContext(nc) as tc, ExitStack() as ctx:
        consts = ctx.enter_context(tc.tile_pool(name="consts", bufs=1))
        ident = consts.tile([P, P], F32)
        make_identity(nc, ident)
        ones_r = consts.tile([1, D], F32R)
        nc.vector.tensor_scalar(ones_r[:], ident[0:1, 0:D], 0.0, 1.0, ALU.mult, ALU.add)

        big = ctx.enter_context(tc.tile_pool(name="big", bufs=12))
        wrp = ctx.enter_context(tc.tile_pool(name="wrp", bufs=1))
        vap = ctx.enter_context(tc.tile_pool(name="vap", bufs=1))
        work = ctx.enter_context(tc.tile_pool(name="work", bufs=3))
        outp = ctx.enter_context(tc.tile_pool(name="outp", bufs=2))
        norm = ctx.enter_context(tc.tile_pool(name="norm", bufs=1))
        pools = ((ident, ones_r), big, wrp, vap, work, outp, norm)
        for rep in range(repeat):
            emit(f"r{rep}_", tc, pools)

    nc.compile()
    return nc


def kernel(X, Wq, Wk, Wv, Wo, bo):
    from concourse import bass_utils

    if "nc" not in _cache:
        _cache["nc"] = _build(int(os.environ.get("KERNEL_REPEAT", "1")))
    nc = _cache["nc"]

    X = np.asarray(X, dtype=np.float32)
    in_maps = []
    for c in range(8):
        b, g = divmod(c, 2)
        sl = slice(HD * g, HD * (g + 1))
        in_maps.append({
            "x": np.ascontiguousarray(X[b]),
            "wq": np.ascontiguousarray(np.asarray(Wq, np.float32)[:, sl]),
            "wk": np.ascontiguousarray(np.asarray(Wk, np.float32)[:, sl]),
            "wv": np.ascontiguousarray(np.asarray(Wv, np.float32)[:, sl]),
            "wo": np.ascontiguousarray(np.asarray(Wo, np.float32)[sl, :]),
        })
    res = bass_utils.run_bass_kernel_spmd(nc, in_maps, core_ids=list(range(8)))
    outf = np.empty((4, T, C), np.float32)
    bo = np.asarray(bo, np.float32)
    for b in range(4):
        outf[b] = res.results[2 * b]["out"] + res.results[2 * b + 1]["out"] + bo
    return outf
